# revision 12
# baseline (speedup 1.0000x reference)
"""Trainium2 Bass kernel for nn_AttenModFullXL: 8-core data-parallel over batch.

v2: DMA batching + dual HWDGE queues, Pool-engine offload for pools/evictions,
PE transposes (no DMA transposes), fused expand bias via identity matmul,
batched softmax + fast reciprocal, collapsed XL-LN as FO post-scale with a
rank-1 mean-correction matmul, s-paired fl1 with one weight DMA per module.

Key structural facts exploited:
  - XL attention outputs are discarded by the reference (dead code) -> skipped.
  - Double XL LayerNorm collapses to one center + per-row scalar scale (exact):
    scale = 4*rsqrt((16+4e)*var + e^2), applied to fl1 partials per batch row.
  - conv1/conv2 run as block-diagonal matmuls over 4-image groups.
  - All matmul inputs bf16 (fp32 accumulate); elementwise/LN/softmax fp32.
"""
import sys, os
sys.path.insert(0, '/root/.axon_site/_ro/trn_rl_repo')
import numpy as np
import ml_dtypes

import concourse.bass as bass
import concourse.mybir as mybir
import concourse.tile as tile
from concourse import bacc
from concourse.bass_utils import run_bass_kernel_spmd
from concourse.masks import make_identity
from contextlib import ExitStack

F32 = mybir.dt.float32
BF16 = mybir.dt.bfloat16
AF = mybir.ActivationFunctionType
ALU = mybir.AluOpType
BF = ml_dtypes.bfloat16
EPS = 1e-5


# ---------------------------------------------------------------- host prep
def host_prep(inputs, c):
    t = np.asarray(inputs['t'], np.float32)
    bs = t.shape[0]
    patches = t.reshape(bs, 4, 32, 4, 32).transpose(0, 1, 3, 2, 4).reshape(bs, 16, 32, 32)
    bl = np.arange(32)
    m_i = np.arange(16)
    bidx = (16 * m_i[:, None] + 2 * c + (bl[None, :] >> 4)).reshape(-1)
    pidx = np.tile(bl & 15, 16)
    imgs = patches[bidx, pidx]                     # (512, 32, 32) f_local = m*32+bl
    sw = np.lib.stride_tricks.sliding_window_view(imgs, (3, 3), axis=(1, 2))
    i2c = sw.transpose(0, 3, 4, 1, 2).reshape(512, 9, 900)
    # (64 pairs, 36 = 4img x 9tap, 2 groups, 900): 2 groups side-by-side per row
    i2c = i2c.reshape(64, 2, 36, 900).transpose(0, 2, 1, 3)
    return {'IM2C': np.ascontiguousarray(i2c).astype(BF)}


def host_prep_shared(inputs):
    d = {}
    w1 = np.asarray(inputs['conv1_w'], np.float32) / 255.0
    # rows = 9i+tap (duplicated at partition 64), cols = 32i+ch
    W1BD = np.zeros((128, 128), np.float32)
    wt = w1[:, 0].reshape(32, 9).T                 # (9 taps, 32 ch)
    for i in range(4):
        W1BD[9*i:9*i+9, 32*i:32*i+32] = wt
        W1BD[64+9*i:64+9*i+9, 32*i:32*i+32] = wt
    d['W1BD'] = W1BD.astype(BF)

    w2 = np.asarray(inputs['conv2_w'], np.float32)
    W2BD = np.zeros((9, 128, 128), np.float32)
    for tap in range(9):
        blk = w2[:, :, tap // 3, tap % 3].T
        for i in range(4):
            W2BD[tap, 32*i:32*i+32, 32*i:32*i+32] = blk
    d['W2BD'] = W2BD.astype(BF)

    expw = np.asarray(inputs['exp_w'], np.float32)
    d['EXPW'] = expw.astype(BF)
    b2_eff = np.asarray(inputs['conv2_b'], np.float32) + \
        np.asarray(inputs['conv1_b'], np.float32) @ w2.sum(axis=(2, 3)).T
    expb_eff = np.asarray(inputs['exp_b'], np.float32)[None, :] + \
        b2_eff[:, None] * expw.sum(axis=0)[None, :]
    d['EXPB'] = expb_eff.astype(BF)                # (32 ch, 64) stationary for bias mm
    IDC = np.zeros((32, 128), np.float32)
    for i in range(4):
        IDC[:, 32*i:32*i+32] = np.eye(32)
    d['IDC'] = IDC.astype(BF)

    wq = np.asarray(inputs['mh_wq'], np.float32); bq = np.asarray(inputs['mh_bq'], np.float32)
    wk = np.asarray(inputs['mh_wk'], np.float32); bk = np.asarray(inputs['mh_bk'], np.float32)
    wv = np.asarray(inputs['mh_wv'], np.float32); bv = np.asarray(inputs['mh_bv'], np.float32)
    wo = np.asarray(inputs['mh_wo'], np.float32); bo = np.asarray(inputs['mh_bo'], np.float32)
    LQ = np.zeros((16, 65, 128), np.float32); LK = np.zeros((16, 65, 128), np.float32)
    LV = np.zeros((16, 65, 128), np.float32)
    WOP = np.zeros((16, 128, 64), np.float32)
    for m in range(16):
        for h in range(4):
            LQ[m, :64, 32*h:32*h+16] = wq[m, h] * 0.25
            LK[m, :64, 32*h:32*h+16] = wk[m, h]
            LV[m, :64, 32*h:32*h+16] = wv[m, h]
            LQ[m, 64, 32*h:32*h+16] = bq[m, h] * 0.25
            LK[m, 64, 32*h:32*h+16] = bk[m, h]
            LV[m, 64, 32*h:32*h+16] = bv[m, h]
            WOP[m, 32*h:32*h+16, :] = wo[m, 16*h:16*h+16, :]
    d['LQ'] = LQ.astype(BF); d['LK'] = LK.astype(BF); d['LV'] = LV.astype(BF)
    d['WOP'] = WOP.astype(BF)
    d['BO'] = bo.reshape(16, 64, 1).astype(np.float32)

    d['CEN'] = (np.eye(64, dtype=np.float32) - 1.0/64.0).astype(BF)
    d['MEAN64'] = np.full((64, 1), 1.0/64.0, np.float32).astype(BF)
    d['ONES1'] = np.ones((1, 64), np.float32).astype(BF)
    d['EPS1'] = np.full((1, 1), EPS, np.float32)
    d['M2048N'] = np.full((64, 1), -1.0/2048.0, np.float32).astype(BF)
    d['XLB'] = np.full((32, 1), EPS*EPS/16.0, np.float32)
    d['M2048P'] = np.full((64, 1), 1.0/2048.0, np.float32).astype(BF)

    fl1 = np.asarray(inputs['fl1_w'], np.float32)
    # [m, 64*(s%2)+d, (s//2)*512+f]
    arr = fl1.reshape(16, 16, 2, 64, 512)          # (m, s2, sp, d, f)
    d['FL1W'] = np.ascontiguousarray(arr.transpose(0, 2, 3, 1, 4).reshape(16, 128, 8192)).astype(BF)
    d['W1SUM'] = fl1.reshape(16, 2048, 512).sum(axis=1).reshape(16, 1, 512).astype(BF)
    d['FB1R'] = np.broadcast_to(np.asarray(inputs['fl1_b'], np.float32), (32, 512)).copy()
    d['FW2'] = np.asarray(inputs['fl2_w'], np.float32).reshape(4, 128, 128).astype(BF)
    d['FB2R'] = np.broadcast_to(np.asarray(inputs['fl2_b'], np.float32), (32, 128)).copy()
    d['FW3'] = np.asarray(inputs['fl3_w'], np.float32).astype(BF)
    d['FB3R'] = np.broadcast_to(np.asarray(inputs['fl3_b'], np.float32), (32, 25)).copy()
    return d


CONST_SPECS = [  # name, shape, dtype
    ('W1BD', (128, 128), BF16), ('W2BD', (9, 128, 128), BF16),
    ('EXPW', (36, 64), BF16), ('EXPB', (32, 64), BF16), ('IDC', (32, 128), BF16),
    ('LQ', (16, 65, 128), BF16), ('LK', (16, 65, 128), BF16), ('LV', (16, 65, 128), BF16),
    ('WOP', (16, 128, 64), BF16), ('BO', (16, 64, 1), F32),
    ('CEN', (64, 64), BF16), ('MEAN64', (64, 1), BF16), ('ONES1', (1, 64), BF16),
    ('EPS1', (1, 1), F32), ('M2048N', (64, 1), BF16), ('XLB', (32, 1), F32), ('M2048P', (64, 1), BF16),
    ('W1SUM', (16, 1, 512), BF16),
    ('FB1R', (32, 512), F32), ('FW2', (4, 128, 128), BF16),
    ('FB2R', (32, 128), F32), ('FW3', (128, 25), BF16), ('FB3R', (32, 25), F32),
]

C1P = 1.0 + EPS / 4.0        # scale for collapsed XL-LN rsqrt arg
C2P = EPS * EPS / 16.0


# ---------------------------------------------------------------- device build
def build_nc():
    nc = bacc.Bacc(None)
    im2c_d = nc.dram_tensor("IM2C", [64, 36, 2, 900], BF16, kind="ExternalInput")
    fl1w_d = nc.dram_tensor("FL1W", [16, 128, 8192], BF16, kind="ExternalInput")
    cd = {}
    for name, shape, dt in CONST_SPECS:
        cd[name] = nc.dram_tensor(name, list(shape), dt, kind="ExternalInput")
    out_d = nc.dram_tensor("OUT", [32, 25], F32, kind="ExternalOutput")

    with tile.TileContext(nc) as tc, ExitStack() as top:
        cpool = top.enter_context(tc.tile_pool(name="consts", bufs=1))
        ct = {}
        for name, shape, dt in CONST_SPECS:
            if len(shape) == 3:
                tiles = []
                for k in range(shape[0]):
                    til = cpool.tile(list(shape[1:]), dt, tag=f"c_{name}_{k}")
                    nc.sync.dma_start(out=til, in_=cd[name][k])
                    tiles.append(til)
                ct[name] = tiles
            else:
                til = cpool.tile(list(shape), dt, tag=f"c_{name}")
                nc.sync.dma_start(out=til, in_=cd[name][:, :])
                ct[name] = til
        IDT = cpool.tile([128, 128], BF16, tag="idt")
        make_identity(nc, IDT)
        U_ALL = cpool.tile([65, 16384], BF16, tag="uall")
        nc.vector.memset(U_ALL[64:65, :], 1.0)
        FACC = cpool.tile([32, 512], F32, tag="facc")
        nc.vector.memset(FACC, 0.0)

        # ---------------- stage A: conv stack -> U_ALL
        with ExitStack() as sa:
            rpool = sa.enter_context(tc.tile_pool(name="sa_rhs", bufs=4))
            wpool = sa.enter_context(tc.tile_pool(name="sa_w", bufs=3))
            dpool = sa.enter_context(tc.tile_pool(name="sa_d", bufs=2))
            psA = sa.enter_context(tc.tile_pool(name="psA", bufs=2, space="PSUM"))
            psB = sa.enter_context(tc.tile_pool(name="psB", bufs=2, space="PSUM"))
            psT = sa.enter_context(tc.tile_pool(name="psT", bufs=1, space="PSUM"))
            psE = sa.enter_context(tc.tile_pool(name="psE", bufs=1, space="PSUM"))
            def stage_conv1(rt, base, gi):
                PA = psA.tile([128, 1024], F32, tag="pa")
                rg = rt[base:base+36, gi*900:gi*900+900]
                nc.tensor.matmul(PA[:, 0:480], ct['W1BD'][base:base+36, :], rg[:, 0:480], start=True, stop=True)
                nc.tensor.matmul(PA[:, 512:932], ct['W1BD'][base:base+36, :], rg[:, 480:900], start=True, stop=True)
                return PA

            def stage_conv2(PA):
                # pool1: x-max (copies on scalar, maxes on DVE), then y-max
                M1 = wpool.tile([128, 450], F32, tag="m1")
                r1a = bass.AP(tensor=PA.tensor, offset=PA.offset, ap=[PA.ap[0], [30, 16], [2, 15]])
                r1b = bass.AP(tensor=PA.tensor, offset=PA.offset + 1, ap=[PA.ap[0], [30, 16], [2, 15]])
                m1a = M1[:, 0:240].rearrange("p (a b) -> p a b", b=15)
                nc.scalar.copy(out=m1a, in_=r1a)
                nc.vector.tensor_max(out=m1a, in0=m1a, in1=r1b)
                r2a = bass.AP(tensor=PA.tensor, offset=PA.offset + 512, ap=[PA.ap[0], [30, 14], [2, 15]])
                r2b = bass.AP(tensor=PA.tensor, offset=PA.offset + 513, ap=[PA.ap[0], [30, 14], [2, 15]])
                m1b = M1[:, 240:450].rearrange("p (a b) -> p a b", b=15)
                nc.scalar.copy(out=m1b, in_=r2a)
                nc.vector.tensor_max(out=m1b, in0=m1b, in1=r2b)
                U1 = wpool.tile([128, 225], BF16, tag="u1")
                ya = bass.AP(tensor=M1.tensor, offset=M1.offset, ap=[M1.ap[0], [30, 15], [1, 15]])
                yb = bass.AP(tensor=M1.tensor, offset=M1.offset + 15, ap=[M1.ap[0], [30, 15], [1, 15]])
                nc.vector.tensor_max(out=U1[:, :].rearrange("p (a b) -> p a b", b=15), in0=ya, in1=yb)
                PB = psB.tile([128, 156], F32, tag="pb")
                for tap in range(9):
                    off = (tap // 3) * 15 + (tap % 3)
                    v = bass.AP(tensor=U1.tensor, offset=U1.offset + off,
                                ap=[U1.ap[0], [15, 12], [1, 13]])
                    nc.tensor.matmul(PB, ct['W2BD'][tap], v,
                                     start=(tap == 0), stop=(tap == 8))
                return PB

            def stage_tail(PB, g):
                M2 = wpool.tile([128, 72], F32, tag="m2")
                xa = bass.AP(tensor=PB.tensor, offset=PB.offset, ap=[PB.ap[0], [13, 12], [2, 6]])
                xb = bass.AP(tensor=PB.tensor, offset=PB.offset + 1, ap=[PB.ap[0], [13, 12], [2, 6]])
                m2v = M2[:, :].rearrange("p (a b) -> p a b", b=6)
                nc.scalar.copy(out=m2v, in_=xa)
                nc.vector.tensor_max(out=m2v, in0=m2v, in1=xb)
                U2T = wpool.tile([128, 36], BF16, tag="u2t")
                ka = bass.AP(tensor=M2.tensor, offset=M2.offset, ap=[M2.ap[0], [12, 6], [1, 6]])
                kb = bass.AP(tensor=M2.tensor, offset=M2.offset + 6, ap=[M2.ap[0], [12, 6], [1, 6]])
                nc.vector.tensor_max(out=U2T[:, :].rearrange("p (a b) -> p a b", b=6), in0=ka, in1=kb)
                PT = psT.tile([36, 128], BF16, tag="pt")
                nc.tensor.transpose(PT, U2T, IDT)
                TPS = dpool.tile([36, 128], BF16, tag="tps")
                nc.scalar.copy(out=TPS, in_=PT)
                PE1 = psE.tile([64, 128], F32, tag="pe1")
                nc.tensor.matmul(PE1, ct['EXPW'], TPS, start=True, stop=False)
                nc.tensor.matmul(PE1, ct['EXPB'], ct['IDC'], start=False, stop=True,
                                 skip_group_check=True)
                nc.vector.tensor_scalar_max(out=U_ALL[0:64, g*128:(g+1)*128], in0=PE1, scalar1=0.0)

            # 3-stage software pipeline over the 128 groups
            q1 = []   # (PA, g) awaiting stage_conv2
            q2 = []   # (PB, g) awaiting stage_tail
            for quad in range(32):
                rt = rpool.tile([128, 1800], BF16, tag="im2c")
                nc.sync.dma_start(out=rt[0:36, :], in_=im2c_d[2*quad])
                nc.scalar.dma_start(out=rt[64:100, :], in_=im2c_d[2*quad+1])
                for sub in range(4):
                    g = 4*quad + sub
                    PA = stage_conv1(rt, 64 * (sub // 2), sub % 2)
                    q1.append((PA, g))
                    if len(q1) > 1:
                        PAp, gp = q1.pop(0)
                        q2.append((stage_conv2(PAp), gp))
                    if len(q2) > 1:
                        PBp, gp = q2.pop(0)
                        stage_tail(PBp, gp)
            while q1:
                PAp, gp = q1.pop(0)
                q2.append((stage_conv2(PAp), gp))
            while q2:
                PBp, gp = q2.pop(0)
                stage_tail(PBp, gp)

        # ---------------- stage B: per-module attention + LN + fl1 accumulation
        with ExitStack() as sbk:
            apool = sbk.enter_context(tc.tile_pool(name="sb_sb", bufs=2))
            spool = sbk.enter_context(tc.tile_pool(name="sb_small", bufs=2))
            atpool = sbk.enter_context(tc.tile_pool(name="sb_at", bufs=4))
            w1pool = sbk.enter_context(tc.tile_pool(name="w1t", bufs=2))
            psP = sbk.enter_context(tc.tile_pool(name="psP", bufs=3, space="PSUM"))
            psO = sbk.enter_context(tc.tile_pool(name="psO", bufs=2, space="PSUM"))
            psX = sbk.enter_context(tc.tile_pool(name="psX", bufs=2, space="PSUM"))
            psF = sbk.enter_context(tc.tile_pool(name="psF", bufs=1, space="PSUM"))
            for m in range(16):
                useg = U_ALL[0:64, m*1024:(m+1)*1024]
                # fl1 weights for this module: one big DMA, alternate queues
                WT = w1pool.tile([128, 8192], BF16, tag="w1t")
                weng = nc.sync if (m % 2 == 0) else nc.scalar
                weng.dma_start(out=WT, in_=fl1w_d[m])
                # -------- Q/K projections -> QKT (128, 2048) bf16
                QKT = apool.tile([128, 2048], BF16, tag="qkt")
                for half in range(2):
                    cs = slice(half*512, half*512+512)
                    PQ = psP.tile([128, 512], F32, tag="psp")
                    nc.tensor.matmul(PQ, ct['LQ'][m], U_ALL[:, m*1024+half*512:m*1024+half*512+512],
                                     start=True, stop=True)
                    nc.scalar.copy(out=QKT[:, cs], in_=PQ)
                    PK = psP.tile([128, 512], F32, tag="psp")
                    nc.tensor.matmul(PK, ct['LK'][m], U_ALL[:, m*1024+half*512:m*1024+half*512+512],
                                     start=True, stop=True)
                    nc.vector.tensor_copy(out=QKT[:, 1024+half*512:1024+half*512+512], in_=PK)
                # -------- scores + batched softmax -> A_ALL (128, 1024) bf16
                A_ALL = apool.tile([128, 1024], BF16, tag="aall")
                for half in range(2):
                    SC = psP.tile([128, 512], F32, tag="psp")
                    for bj in range(16):
                        b = half * 16 + bj
                        for h in range(4):
                            nc.tensor.matmul(
                                SC[32*h:32*h+32, bj*32:bj*32+32],
                                QKT[32*h:32*h+16, b*32:b*32+32],
                                QKT[32*h:32*h+16, 1024+b*32:1024+b*32+32],
                                start=True, stop=True, tile_position=(32*h, 32*h))
                    EA = spool.tile([128, 512], F32, tag="ea")
                    nc.scalar.activation(out=EA, in_=SC, func=AF.Exp)
                    SM = spool.tile([128, 16], F32, tag="sm")
                    nc.vector.reduce_sum(out=SM, in_=EA.rearrange("p (a b) -> p a b", b=32),
                                         axis=mybir.AxisListType.X)
                    RS = spool.tile([128, 16], F32, tag="rs")
                    nc.vector.reciprocal_approx_fast(out=RS, in_=SM)
                    rsb = bass.AP(tensor=RS.tensor, offset=RS.offset, ap=[RS.ap[0], [1, 16], [0, 32]])
                    nc.vector.tensor_tensor(
                        out=A_ALL[:, half*512:(half+1)*512].rearrange("p (a b) -> p a b", b=32),
                        in0=EA.rearrange("p (a b) -> p a b", b=32), in1=rsb, op=ALU.mult)
                # -------- V natural + A transpose (PE) + AV + concat -> OT
                OT = apool.tile([128, 1024], BF16, tag="ot")
                for B in range(8):
                    PV = psO.tile([128, 128], F32, tag="pso")
                    for j in range(4):
                        bp = 4*B + j
                        nc.tensor.matmul(PV[32*j:32*j+32, :],
                                         U_ALL[:, m*1024+bp*32:m*1024+bp*32+32], ct['LV'][m],
                                         start=True, stop=True, tile_position=(0, 32*j))
                    VN = atpool.tile([128, 128], BF16, tag="vn")
                    nc.scalar.copy(out=VN, in_=PV)
                    PTA = psO.tile([128, 128], BF16, tag="pso")
                    nc.tensor.transpose(PTA, A_ALL[:, B*128:(B+1)*128], IDT)
                    ATS = atpool.tile([128, 128], BF16, tag="ats")
                    nc.scalar.copy(out=ATS, in_=PTA)
                    for j in range(4):
                        POj = psO.tile([128, 32], F32, tag="pso")
                        for h in range(4):
                            nc.tensor.matmul(
                                POj[32*h:32*h+32, 0:32],
                                VN[32*j:32*j+32, 32*h:32*h+32],
                                ATS[32*j:32*j+32, 32*h:32*h+32],
                                start=True, stop=True, tile_position=(32*j, 32*h),
                                skip_group_check=True)
                        if j % 2 == 0:
                            nc.scalar.copy(out=OT[:, B*128+j*32:B*128+j*32+32], in_=POj)
                        else:
                            nc.vector.tensor_copy(out=OT[:, B*128+j*32:B*128+j*32+32], in_=POj)
                # -------- WO + residual -> XB bf16 (fused)
                XB = apool.tile([64, 1024], BF16, tag="xb")
                for half in range(2):
                    cs = slice(half*512, half*512+512)
                    XO = psX.tile([64, 512], F32, tag="psx")
                    nc.tensor.matmul(XO, ct['WOP'][m], OT[:, cs], start=True, stop=True)
                    nc.vector.scalar_tensor_tensor(out=XB[:, cs], in0=XO, scalar=ct['BO'][m],
                                                   in1=useg[:, cs], op0=ALU.add, op1=ALU.add)
                # -------- LN1 over partitions via centering matmul
                XCS = apool.tile([64, 1024], BF16, tag="xcs")
                XSQ = apool.tile([64, 1024], BF16, tag="xsq")
                SQV = spool.tile([1, 1024], F32, tag="sqv")
                for half in range(2):
                    cs = slice(half*512, half*512+512)
                    XC = psX.tile([64, 512], F32, tag="psx")
                    nc.tensor.matmul(XC, ct['CEN'], XB[:, cs], start=True, stop=True)
                    nc.vector.tensor_copy(out=XCS[:, cs], in_=XC)
                    nc.vector.tensor_mul(out=XSQ[:, cs], in0=XCS[:, cs], in1=XCS[:, cs])
                    V1 = psX.tile([1, 512], F32, tag="psx")
                    nc.tensor.matmul(V1, ct['MEAN64'], XSQ[:, cs], start=True, stop=True)
                    nc.scalar.activation(out=SQV[:, cs], in_=V1, func=AF.Sqrt, bias=ct['EPS1'])
                R1F = spool.tile([1, 1024], F32, tag="r1f")
                nc.vector.reciprocal_approx_fast(out=R1F, in_=SQV)
                R1B = spool.tile([1, 1024], BF16, tag="r1b")
                nc.vector.tensor_copy(out=R1B, in_=R1F)
                # U2D: rows 0-63 = U2M; rows 64-127 = U2M shifted by one s (for s-pair fl1)
                U2D = apool.tile([128, 1024], BF16, tag="u2d")
                for half in range(2):
                    cs = slice(half*512, half*512+512)
                    RR = psX.tile([64, 512], F32, tag="psx")
                    nc.tensor.matmul(RR, ct['ONES1'], R1B[:, cs], start=True, stop=True)
                    nc.vector.tensor_mul(out=U2D[0:64, cs], in0=XCS[:, cs], in1=RR)
                nc.scalar.dma_start(out=U2D[64:128, 0:1023], in_=U2D[0:64, 1:1024])
                # -------- XL double-LN stats (collapsed, single sqrt)
                P1S = spool.tile([64, 32], F32, tag="p1s")
                nc.vector.reduce_sum(out=P1S, in_=U2D[0:64, :].rearrange("p (a b) -> p a b", b=32),
                                     axis=mybir.AxisListType.X)
                USQ = spool.tile([64, 1024], F32, tag="usq")
                nc.vector.tensor_mul(out=USQ, in0=U2D[0:64, :], in1=U2D[0:64, :])
                P2S = spool.tile([64, 32], F32, tag="p2s")
                nc.vector.reduce_sum(out=P2S, in_=USQ.rearrange("p (a b) -> p a b", b=32),
                                     axis=mybir.AxisListType.X)
                P1B = spool.tile([64, 32], BF16, tag="p1b")
                P2B = spool.tile([64, 32], BF16, tag="p2b")
                nc.vector.tensor_copy(out=P1B, in_=P1S)
                nc.vector.tensor_copy(out=P2B, in_=P2S)
                STP = psX.tile([32, 2], F32, tag="psx")
                nc.tensor.matmul(STP[:, 0:1], P1B, ct['M2048N'], start=True, stop=True)
                nc.tensor.matmul(STP[:, 1:2], P2B, ct['M2048P'], start=True, stop=True,
                                 skip_group_check=True)
                MUP = psX.tile([1, 32], F32, tag="psx")
                nc.tensor.matmul(MUP, ct['M2048N'], P1B, start=True, stop=True)
                MUNB = spool.tile([1, 32], BF16, tag="munb")
                nc.vector.tensor_copy(out=MUNB, in_=MUP)
                STB = spool.tile([32, 2], F32, tag="stb")
                nc.vector.tensor_copy(out=STB, in_=STP)
                MM2 = spool.tile([32, 1], F32, tag="mm2")
                nc.vector.tensor_mul(out=MM2, in0=STB[:, 0:1], in1=STB[:, 0:1])
                V32 = spool.tile([32, 1], F32, tag="v32")
                nc.vector.tensor_sub(out=V32, in0=STB[:, 1:2], in1=MM2)
                SQ32 = spool.tile([32, 1], F32, tag="sq32")
                nc.scalar.activation(out=SQ32, in_=V32, func=AF.Sqrt, bias=ct['XLB'])
                S4 = spool.tile([32, 1], F32, tag="s4")
                nc.vector.reciprocal_approx_fast(out=S4, in_=SQ32)
                # -------- fl1 partial: 16 s-paired matmuls + rank-1 mean correction
                FO = psF.tile([32, 512], F32, tag="fo")
                for k in range(16):
                    lhs = bass.AP(tensor=U2D.tensor, offset=U2D.offset + 2*k, ap=[U2D.ap[0], [32, 32]])
                    nc.tensor.matmul(FO, lhs, WT[:, k*512:(k+1)*512], start=(k == 0), stop=False)
                nc.tensor.matmul(FO, MUNB, ct['W1SUM'][m], start=False, stop=True)
                nc.vector.scalar_tensor_tensor(out=FACC, in0=FO, scalar=S4, in1=FACC,
                                               op0=ALU.mult, op1=ALU.add)

            # ---------------- final MLP
            O1B = spool.tile([32, 512], BF16, tag="o1b")
            T5 = spool.tile([32, 512], F32, tag="t5")
            nc.vector.tensor_add(out=T5, in0=FACC, in1=ct['FB1R'])
            nc.vector.tensor_scalar_max(out=O1B, in0=T5, scalar1=0.0)
            F2 = psX.tile([32, 128], F32, tag="psx")
            for k in range(4):
                PL2 = psO.tile([128, 32], BF16, tag="pso")
                nc.tensor.transpose(PL2, O1B[:, k*128:(k+1)*128], IDT[0:32, 0:32])
                L2 = spool.tile([128, 32], BF16, tag="l2")
                nc.scalar.copy(out=L2, in_=PL2)
                nc.tensor.matmul(F2, L2, ct['FW2'][k], start=(k == 0), stop=(k == 3))
            O2B = spool.tile([32, 128], BF16, tag="o2b")
            T6 = spool.tile([32, 128], F32, tag="t6")
            nc.vector.tensor_add(out=T6, in0=F2, in1=ct['FB2R'])
            nc.vector.tensor_scalar_max(out=O2B, in0=T6, scalar1=0.0)
            PL3 = psO.tile([128, 32], BF16, tag="pso")
            nc.tensor.transpose(PL3, O2B, IDT[0:32, 0:32])
            L3 = spool.tile([128, 32], BF16, tag="l3")
            nc.scalar.copy(out=L3, in_=PL3)
            F3 = psX.tile([32, 25], F32, tag="psx")
            nc.tensor.matmul(F3, L3, ct['FW3'], start=True, stop=True)
            OUTS = spool.tile([32, 25], F32, tag="outs")
            nc.vector.tensor_add(out=OUTS, in0=F3, in1=ct['FB3R'])
            nc.gpsimd.dma_start(out=out_d[:, :], in_=OUTS)

    nc.compile()
    return nc


_NC_CACHE = None

def _get_nc():
    global _NC_CACHE
    if _NC_CACHE is None:
        _NC_CACHE = build_nc()
    return _NC_CACHE


def kernel(**inputs):
    nc = _get_nc()
    shared = host_prep_shared(inputs)
    in_maps = []
    for c in range(8):
        d = dict(shared)
        d.update(host_prep(inputs, c))
        in_maps.append(d)
    res = run_bass_kernel_spmd(nc, in_maps, list(range(8)))
    out = np.concatenate([res.results[c]["OUT"] for c in range(8)], axis=0)
    return out.astype(np.float32)


if __name__ == '__main__':
    import pickle
    with open('/tmp/refdata.pkl', 'rb') as f:
        inputs, expected = pickle.load(f)
    out = kernel(**inputs)
    rv = float(((out - expected)**2).mean() / (expected**2).mean())
    print("kernel resid_var:", rv)


# revision 14
# speedup vs baseline: 1.1736x; 1.1736x over previous
"""Trainium2 Bass kernel for nn_AttenModFullXL: 8-core data-parallel over batch.

v2: DMA batching + dual HWDGE queues, Pool-engine offload for pools/evictions,
PE transposes (no DMA transposes), fused expand bias via identity matmul,
batched softmax + fast reciprocal, collapsed XL-LN as FO post-scale with a
rank-1 mean-correction matmul, s-paired fl1 with one weight DMA per module.

Key structural facts exploited:
  - XL attention outputs are discarded by the reference (dead code) -> skipped.
  - Double XL LayerNorm collapses to one center + per-row scalar scale (exact):
    scale = 4*rsqrt((16+4e)*var + e^2), applied to fl1 partials per batch row.
  - conv1/conv2 run as block-diagonal matmuls over 4-image groups.
  - All matmul inputs bf16 (fp32 accumulate); elementwise/LN/softmax fp32.
"""
import sys, os
sys.path.insert(0, '/root/.axon_site/_ro/trn_rl_repo')
import numpy as np
import ml_dtypes

import concourse.bass as bass
import concourse.mybir as mybir
import concourse.tile as tile
from concourse import bacc
from concourse.bass_utils import run_bass_kernel_spmd
from concourse.masks import make_identity
from contextlib import ExitStack

F32 = mybir.dt.float32
BF16 = mybir.dt.bfloat16
AF = mybir.ActivationFunctionType
ALU = mybir.AluOpType
BF = ml_dtypes.bfloat16
EPS = 1e-5


# ---------------------------------------------------------------- host prep
def host_prep(inputs, c):
    t = np.asarray(inputs['t'], np.float32)
    bs = t.shape[0]
    patches = t.reshape(bs, 4, 32, 4, 32).transpose(0, 1, 3, 2, 4).reshape(bs, 16, 32, 32)
    bl = np.arange(32)
    m_i = np.arange(16)
    bidx = (16 * m_i[:, None] + 2 * c + (bl[None, :] >> 4)).reshape(-1)
    pidx = np.tile(bl & 15, 16)
    imgs = patches[bidx, pidx]                     # (512, 32, 32) f_local = m*32+bl
    sw = np.lib.stride_tricks.sliding_window_view(imgs, (3, 3), axis=(1, 2))
    i2c = sw.transpose(0, 3, 4, 1, 2).reshape(512, 9, 900)
    # (64 pairs, 36 = 4img x 9tap, 2 groups, 900): 2 groups side-by-side per row
    i2c = i2c.reshape(64, 2, 36, 900).transpose(0, 2, 1, 3)
    return {'IM2C': np.ascontiguousarray(i2c).astype(BF)}


def host_prep_shared(inputs):
    d = {}
    w1 = np.asarray(inputs['conv1_w'], np.float32) / 255.0
    # rows = 9i+tap (duplicated at partition 64), cols = 32i+ch
    W1BD = np.zeros((128, 128), np.float32)
    wt = w1[:, 0].reshape(32, 9).T                 # (9 taps, 32 ch)
    for i in range(4):
        W1BD[9*i:9*i+9, 32*i:32*i+32] = wt
        W1BD[64+9*i:64+9*i+9, 32*i:32*i+32] = wt
    d['W1BD'] = W1BD.astype(BF)

    w2 = np.asarray(inputs['conv2_w'], np.float32)
    W2BD = np.zeros((9, 128, 128), np.float32)
    for tap in range(9):
        blk = w2[:, :, tap // 3, tap % 3].T
        for i in range(4):
            W2BD[tap, 32*i:32*i+32, 32*i:32*i+32] = blk
    d['W2BD'] = W2BD.astype(BF)

    expw = np.asarray(inputs['exp_w'], np.float32)
    d['EXPW'] = expw.astype(BF)
    b2_eff = np.asarray(inputs['conv2_b'], np.float32) + \
        np.asarray(inputs['conv1_b'], np.float32) @ w2.sum(axis=(2, 3)).T
    expb_eff = np.asarray(inputs['exp_b'], np.float32)[None, :] + \
        b2_eff[:, None] * expw.sum(axis=0)[None, :]
    d['EXPB'] = expb_eff.astype(BF)                # (32 ch, 64) stationary for bias mm
    IDC = np.zeros((32, 128), np.float32)
    for i in range(4):
        IDC[:, 32*i:32*i+32] = np.eye(32)
    d['IDC'] = IDC.astype(BF)

    wq = np.asarray(inputs['mh_wq'], np.float32); bq = np.asarray(inputs['mh_bq'], np.float32)
    wk = np.asarray(inputs['mh_wk'], np.float32); bk = np.asarray(inputs['mh_bk'], np.float32)
    wv = np.asarray(inputs['mh_wv'], np.float32); bv = np.asarray(inputs['mh_bv'], np.float32)
    wo = np.asarray(inputs['mh_wo'], np.float32); bo = np.asarray(inputs['mh_bo'], np.float32)
    LQ = np.zeros((16, 65, 128), np.float32); LK = np.zeros((16, 65, 128), np.float32)
    LV = np.zeros((16, 65, 128), np.float32)
    WOP = np.zeros((16, 128, 64), np.float32)
    for m in range(16):
        for h in range(4):
            LQ[m, :64, 32*h:32*h+16] = wq[m, h] * 0.25
            LK[m, :64, 32*h:32*h+16] = wk[m, h]
            LV[m, :64, 32*h:32*h+16] = wv[m, h]
            LQ[m, 64, 32*h:32*h+16] = bq[m, h] * 0.25
            LK[m, 64, 32*h:32*h+16] = bk[m, h]
            LV[m, 64, 32*h:32*h+16] = bv[m, h]
            WOP[m, 32*h:32*h+16, :] = wo[m, 16*h:16*h+16, :]
    d['LQ'] = LQ.astype(BF); d['LK'] = LK.astype(BF); d['LV'] = LV.astype(BF)
    d['WOP'] = WOP.astype(BF)
    d['BO'] = bo.reshape(16, 64, 1).astype(np.float32)

    d['CEN'] = (np.eye(64, dtype=np.float32) - 1.0/64.0).astype(BF)
    d['MEAN64'] = np.full((64, 1), 1.0/64.0, np.float32).astype(BF)
    d['ONES1'] = np.ones((1, 64), np.float32).astype(BF)
    d['EPS1'] = np.full((1, 1), EPS, np.float32)
    d['M2048N'] = np.full((64, 1), -1.0/2048.0, np.float32).astype(BF)
    d['XLB'] = np.full((32, 1), EPS*EPS/16.0, np.float32)
    d['M2048P'] = np.full((64, 1), 1.0/2048.0, np.float32).astype(BF)

    fl1 = np.asarray(inputs['fl1_w'], np.float32)
    # [m, 64*(s%2)+d, (s//2)*512+f]
    arr = fl1.reshape(16, 16, 2, 64, 512)          # (m, s2, sp, d, f)
    d['FL1W'] = np.ascontiguousarray(arr.transpose(0, 2, 3, 1, 4).reshape(16, 128, 8192)).astype(BF)
    d['W1SUM'] = fl1.reshape(16, 2048, 512).sum(axis=1).reshape(16, 1, 512).astype(BF)
    d['FB1R'] = np.broadcast_to(np.asarray(inputs['fl1_b'], np.float32), (32, 512)).copy()
    d['FW2'] = np.asarray(inputs['fl2_w'], np.float32).reshape(4, 128, 128).astype(BF)
    d['FB2R'] = np.broadcast_to(np.asarray(inputs['fl2_b'], np.float32), (32, 128)).copy()
    d['FW3'] = np.asarray(inputs['fl3_w'], np.float32).astype(BF)
    d['FB3R'] = np.broadcast_to(np.asarray(inputs['fl3_b'], np.float32), (32, 25)).copy()
    return d


CONST_SPECS = [  # name, shape, dtype
    ('W1BD', (128, 128), BF16), ('W2BD', (9, 128, 128), BF16),
    ('EXPW', (36, 64), BF16), ('EXPB', (32, 64), BF16), ('IDC', (32, 128), BF16),
    ('LQ', (16, 65, 128), BF16), ('LK', (16, 65, 128), BF16), ('LV', (16, 65, 128), BF16),
    ('WOP', (16, 128, 64), BF16), ('BO', (16, 64, 1), F32),
    ('CEN', (64, 64), BF16), ('MEAN64', (64, 1), BF16), ('ONES1', (1, 64), BF16),
    ('EPS1', (1, 1), F32), ('M2048N', (64, 1), BF16), ('XLB', (32, 1), F32), ('M2048P', (64, 1), BF16),
    ('W1SUM', (16, 1, 512), BF16),
    ('FB1R', (32, 512), F32), ('FW2', (4, 128, 128), BF16),
    ('FB2R', (32, 128), F32), ('FW3', (128, 25), BF16), ('FB3R', (32, 25), F32),
]

C1P = 1.0 + EPS / 4.0        # scale for collapsed XL-LN rsqrt arg
C2P = EPS * EPS / 16.0


# ---------------------------------------------------------------- device build
def build_nc():
    nc = bacc.Bacc(None)
    im2c_d = nc.dram_tensor("IM2C", [64, 36, 2, 900], BF16, kind="ExternalInput")
    fl1w_d = nc.dram_tensor("FL1W", [16, 128, 8192], BF16, kind="ExternalInput")
    cd = {}
    for name, shape, dt in CONST_SPECS:
        cd[name] = nc.dram_tensor(name, list(shape), dt, kind="ExternalInput")
    out_d = nc.dram_tensor("OUT", [32, 25], F32, kind="ExternalOutput")

    with tile.TileContext(nc) as tc, ExitStack() as top:
        cpool = top.enter_context(tc.tile_pool(name="consts", bufs=1))
        CONV_CONSTS = {'W1BD', 'W2BD', 'EXPW', 'EXPB', 'IDC'}
        ct = {}
        deferred = []
        qtoggle = [0]
        def load_const(name, shape, dt):
            if len(shape) == 3:
                tiles = []
                for k in range(shape[0]):
                    til = cpool.tile(list(shape[1:]), dt, tag=f"c_{name}_{k}", name=f"c_{name}_{k}")
                    eng = nc.sync if qtoggle[0] % 2 == 0 else nc.scalar
                    qtoggle[0] += 1
                    eng.dma_start(out=til, in_=cd[name][k])
                    tiles.append(til)
                ct[name] = tiles
            else:
                til = cpool.tile(list(shape), dt, tag=f"c_{name}", name=f"c_{name}")
                eng = nc.sync if qtoggle[0] % 2 == 0 else nc.scalar
                qtoggle[0] += 1
                eng.dma_start(out=til, in_=cd[name][:, :])
                ct[name] = til
        for name, shape, dt in CONST_SPECS:
            if name in CONV_CONSTS:
                load_const(name, shape, dt)
            else:
                deferred.append((name, shape, dt))
        IDT = cpool.tile([128, 128], BF16, tag="idt")
        make_identity(nc, IDT)
        U_ALL = cpool.tile([65, 16384], BF16, tag="uall")
        nc.vector.memset(U_ALL[64:65, :], 1.0)
        FACC = cpool.tile([32, 512], F32, tag="facc")
        nc.vector.memset(FACC, 0.0)

        # ---------------- stage A: conv stack -> U_ALL
        with ExitStack() as sa:
            rpool = sa.enter_context(tc.tile_pool(name="sa_rhs", bufs=4))
            wpool = sa.enter_context(tc.tile_pool(name="sa_w", bufs=3))
            dpool = sa.enter_context(tc.tile_pool(name="sa_d", bufs=2))
            psA = sa.enter_context(tc.tile_pool(name="psA", bufs=2, space="PSUM"))
            psB = sa.enter_context(tc.tile_pool(name="psB", bufs=2, space="PSUM"))
            psT = sa.enter_context(tc.tile_pool(name="psT", bufs=1, space="PSUM"))
            psE = sa.enter_context(tc.tile_pool(name="psE", bufs=1, space="PSUM"))
            def stage_conv1(rt, base, gi):
                PA = psA.tile([128, 1024], F32, tag="pa")
                rg = rt[base:base+36, gi*900:gi*900+900]
                nc.tensor.matmul(PA[:, 0:480], ct['W1BD'][base:base+36, :], rg[:, 0:480], start=True, stop=True)
                nc.tensor.matmul(PA[:, 512:932], ct['W1BD'][base:base+36, :], rg[:, 480:900], start=True, stop=True)
                return PA

            def stage_conv2(PA):
                # pool1: x-max (copies on scalar, maxes on DVE), then y-max
                M1 = wpool.tile([128, 450], F32, tag="m1")
                r1a = bass.AP(tensor=PA.tensor, offset=PA.offset, ap=[PA.ap[0], [30, 16], [2, 15]])
                r1b = bass.AP(tensor=PA.tensor, offset=PA.offset + 1, ap=[PA.ap[0], [30, 16], [2, 15]])
                m1a = M1[:, 0:240].rearrange("p (a b) -> p a b", b=15)
                nc.scalar.copy(out=m1a, in_=r1a)
                nc.vector.tensor_max(out=m1a, in0=m1a, in1=r1b)
                r2a = bass.AP(tensor=PA.tensor, offset=PA.offset + 512, ap=[PA.ap[0], [30, 14], [2, 15]])
                r2b = bass.AP(tensor=PA.tensor, offset=PA.offset + 513, ap=[PA.ap[0], [30, 14], [2, 15]])
                m1b = M1[:, 240:450].rearrange("p (a b) -> p a b", b=15)
                nc.scalar.copy(out=m1b, in_=r2a)
                nc.vector.tensor_max(out=m1b, in0=m1b, in1=r2b)
                U1 = wpool.tile([128, 225], BF16, tag="u1")
                ya = bass.AP(tensor=M1.tensor, offset=M1.offset, ap=[M1.ap[0], [30, 15], [1, 15]])
                yb = bass.AP(tensor=M1.tensor, offset=M1.offset + 15, ap=[M1.ap[0], [30, 15], [1, 15]])
                nc.vector.tensor_max(out=U1[:, :].rearrange("p (a b) -> p a b", b=15), in0=ya, in1=yb)
                PB = psB.tile([128, 156], F32, tag="pb")
                for tap in range(9):
                    off = (tap // 3) * 15 + (tap % 3)
                    v = bass.AP(tensor=U1.tensor, offset=U1.offset + off,
                                ap=[U1.ap[0], [15, 12], [1, 13]])
                    nc.tensor.matmul(PB, ct['W2BD'][tap], v,
                                     start=(tap == 0), stop=(tap == 8))
                return PB

            def stage_tail(PB, g):
                M2 = wpool.tile([128, 72], F32, tag="m2")
                xa = bass.AP(tensor=PB.tensor, offset=PB.offset, ap=[PB.ap[0], [13, 12], [2, 6]])
                xb = bass.AP(tensor=PB.tensor, offset=PB.offset + 1, ap=[PB.ap[0], [13, 12], [2, 6]])
                m2v = M2[:, :].rearrange("p (a b) -> p a b", b=6)
                nc.scalar.copy(out=m2v, in_=xa)
                nc.vector.tensor_max(out=m2v, in0=m2v, in1=xb)
                U2T = wpool.tile([128, 36], BF16, tag="u2t")
                ka = bass.AP(tensor=M2.tensor, offset=M2.offset, ap=[M2.ap[0], [12, 6], [1, 6]])
                kb = bass.AP(tensor=M2.tensor, offset=M2.offset + 6, ap=[M2.ap[0], [12, 6], [1, 6]])
                nc.vector.tensor_max(out=U2T[:, :].rearrange("p (a b) -> p a b", b=6), in0=ka, in1=kb)
                PT = psT.tile([36, 128], BF16, tag="pt")
                nc.tensor.transpose(PT, U2T, IDT)
                TPS = dpool.tile([36, 128], BF16, tag="tps")
                nc.scalar.copy(out=TPS, in_=PT)
                PE1 = psE.tile([64, 128], F32, tag="pe1")
                nc.tensor.matmul(PE1, ct['EXPW'], TPS, start=True, stop=False)
                nc.tensor.matmul(PE1, ct['EXPB'], ct['IDC'], start=False, stop=True,
                                 skip_group_check=True)
                nc.vector.tensor_scalar_max(out=U_ALL[0:64, g*128:(g+1)*128], in0=PE1, scalar1=0.0)

            # 3-stage software pipeline over the 128 groups
            q1 = []   # (PA, g) awaiting stage_conv2
            q2 = []   # (PB, g) awaiting stage_tail
            for quad in range(32):
                rt = rpool.tile([128, 1800], BF16, tag="im2c")
                nc.sync.dma_start(out=rt[0:36, :], in_=im2c_d[2*quad])
                nc.scalar.dma_start(out=rt[64:100, :], in_=im2c_d[2*quad+1])
                for sub in range(4):
                    g = 4*quad + sub
                    PA = stage_conv1(rt, 64 * (sub // 2), sub % 2)
                    q1.append((PA, g))
                    if len(q1) > 1:
                        PAp, gp = q1.pop(0)
                        q2.append((stage_conv2(PAp), gp))
                    if len(q2) > 1:
                        PBp, gp = q2.pop(0)
                        stage_tail(PBp, gp)
            while q1:
                PAp, gp = q1.pop(0)
                q2.append((stage_conv2(PAp), gp))
            while q2:
                PBp, gp = q2.pop(0)
                stage_tail(PBp, gp)
            for name, shape, dt in deferred:
                load_const(name, shape, dt)

        # ---------------- stage B: per-module attention + LN + fl1 accumulation
        with ExitStack() as sbk:
            apool = sbk.enter_context(tc.tile_pool(name="sb_sb", bufs=2))
            spool = sbk.enter_context(tc.tile_pool(name="sb_small", bufs=2))
            atpool = sbk.enter_context(tc.tile_pool(name="sb_at", bufs=4))
            w1pool = sbk.enter_context(tc.tile_pool(name="w1t", bufs=2))
            psP = sbk.enter_context(tc.tile_pool(name="psP", bufs=2, space="PSUM"))
            psO = sbk.enter_context(tc.tile_pool(name="psO", bufs=3, space="PSUM"))
            psX = sbk.enter_context(tc.tile_pool(name="psX", bufs=2, space="PSUM"))
            psF = sbk.enter_context(tc.tile_pool(name="psF", bufs=1, space="PSUM"))
            def stageS1(m):
                st = {}
                WT = w1pool.tile([128, 8192], BF16, tag="w1t", bufs=3, name="WT")
                weng = nc.sync if (m % 2 == 0) else nc.scalar
                weng.dma_start(out=WT, in_=fl1w_d[m])
                st['WT'] = WT
                QKT = apool.tile([128, 2048], BF16, tag="qkt", name="QKT")
                for half in range(2):
                    cs = slice(half*512, half*512+512)
                    PQ = psP.tile([128, 512], F32, tag="psp", name="PQ")
                    nc.tensor.matmul(PQ, ct['LQ'][m], U_ALL[:, m*1024+half*512:m*1024+half*512+512],
                                     start=True, stop=True)
                    nc.scalar.copy(out=QKT[:, cs], in_=PQ)
                    PK = psP.tile([128, 512], F32, tag="psp", name="PK")
                    nc.tensor.matmul(PK, ct['LK'][m], U_ALL[:, m*1024+half*512:m*1024+half*512+512],
                                     start=True, stop=True)
                    nc.vector.tensor_copy(out=QKT[:, 1024+half*512:1024+half*512+512], in_=PK)
                eas = []
                for half in range(2):
                    SC = psP.tile([128, 512], F32, tag="psp", name="SC")
                    for bj in range(16):
                        b = half * 16 + bj
                        for h in range(4):
                            nc.tensor.matmul(
                                SC[32*h:32*h+32, bj*32:bj*32+32],
                                QKT[32*h:32*h+16, b*32:b*32+32],
                                QKT[32*h:32*h+16, 1024+b*32:1024+b*32+32],
                                start=True, stop=True, tile_position=(32*h, 32*h))
                    EA = spool.tile([128, 512], F32, tag="ea", bufs=4, name="EA")
                    nc.scalar.activation(out=EA, in_=SC, func=AF.Exp)
                    eas.append(EA)
                st['eas'] = eas
                return st

            def stageS2(m, st):
                A_ALL = apool.tile([128, 1024], BF16, tag="aall", name="A_ALL")
                for half in range(2):
                    EA = st['eas'][half]
                    SM = spool.tile([128, 16], F32, tag="sm", name="SM")
                    nc.vector.reduce_sum(out=SM, in_=EA.rearrange("p (a b) -> p a b", b=32),
                                         axis=mybir.AxisListType.X)
                    RS = spool.tile([128, 16], F32, tag="rs", name="RS")
                    nc.vector.reciprocal_approx_fast(out=RS, in_=SM)
                    rsb = bass.AP(tensor=RS.tensor, offset=RS.offset, ap=[RS.ap[0], [1, 16], [0, 32]])
                    nc.vector.tensor_tensor(
                        out=A_ALL[:, half*512:(half+1)*512].rearrange("p (a b) -> p a b", b=32),
                        in0=EA.rearrange("p (a b) -> p a b", b=32), in1=rsb, op=ALU.mult)
                st['A_ALL'] = A_ALL

            def stageS3(m, st):
                A_ALL = st['A_ALL']
                OT = apool.tile([128, 1024], BF16, tag="ot", name="OT")
                for B in range(8):
                    PV = psO.tile([128, 128], F32, tag="pso", name="PV")
                    for j in range(4):
                        bp = 4*B + j
                        nc.tensor.matmul(PV[32*j:32*j+32, :],
                                         U_ALL[:, m*1024+bp*32:m*1024+bp*32+32], ct['LV'][m],
                                         start=True, stop=True, tile_position=(0, 32*j))
                    VN = atpool.tile([128, 128], BF16, tag="vn", name="VN")
                    nc.scalar.copy(out=VN, in_=PV)
                    PTA = psO.tile([128, 128], BF16, tag="pso", name="PTA")
                    nc.tensor.transpose(PTA, A_ALL[:, B*128:(B+1)*128], IDT)
                    ATS = atpool.tile([128, 128], BF16, tag="ats", name="ATS")
                    nc.scalar.copy(out=ATS, in_=PTA)
                    for j in range(4):
                        POj = psO.tile([128, 32], F32, tag="pso", name="POj")
                        for h in range(4):
                            nc.tensor.matmul(
                                POj[32*h:32*h+32, 0:32],
                                VN[32*j:32*j+32, 32*h:32*h+32],
                                ATS[32*j:32*j+32, 32*h:32*h+32],
                                start=True, stop=True, tile_position=(32*j, 32*h),
                                skip_group_check=True)
                        if j % 2 == 0:
                            nc.scalar.copy(out=OT[:, B*128+j*32:B*128+j*32+32], in_=POj)
                        else:
                            nc.vector.tensor_copy(out=OT[:, B*128+j*32:B*128+j*32+32], in_=POj)
                st['OT'] = OT

            def stageS4(m, st):
                useg = U_ALL[0:64, m*1024:(m+1)*1024]
                OT = st['OT']
                XB = apool.tile([64, 1024], BF16, tag="xb", name="XB")
                for half in range(2):
                    cs = slice(half*512, half*512+512)
                    XO = psX.tile([64, 512], F32, tag="psx", name="XO")
                    nc.tensor.matmul(XO, ct['WOP'][m], OT[:, cs], start=True, stop=True)
                    nc.vector.scalar_tensor_tensor(out=XB[:, cs], in0=XO, scalar=ct['BO'][m],
                                                   in1=useg[:, cs], op0=ALU.add, op1=ALU.add)
                XCS = apool.tile([64, 1024], BF16, tag="xcs", name="XCS")
                XSQ = apool.tile([64, 1024], BF16, tag="xsq", name="XSQ")
                SQV = spool.tile([1, 1024], F32, tag="sqv", name="SQV")
                for half in range(2):
                    cs = slice(half*512, half*512+512)
                    XC = psX.tile([64, 512], F32, tag="psx", name="XC")
                    nc.tensor.matmul(XC, ct['CEN'], XB[:, cs], start=True, stop=True)
                    nc.vector.tensor_copy(out=XCS[:, cs], in_=XC)
                    nc.vector.tensor_mul(out=XSQ[:, cs], in0=XCS[:, cs], in1=XCS[:, cs])
                    V1 = psX.tile([1, 512], F32, tag="psx", name="V1")
                    nc.tensor.matmul(V1, ct['MEAN64'], XSQ[:, cs], start=True, stop=True)
                    nc.scalar.activation(out=SQV[:, cs], in_=V1, func=AF.Sqrt, bias=ct['EPS1'])
                R1F = spool.tile([1, 1024], F32, tag="r1f", name="R1F")
                nc.vector.reciprocal_approx_fast(out=R1F, in_=SQV)
                R1B = spool.tile([1, 1024], BF16, tag="r1b", name="R1B")
                nc.vector.tensor_copy(out=R1B, in_=R1F)
                U2D = apool.tile([128, 1024], BF16, tag="u2d", bufs=3, name="U2D")
                for half in range(2):
                    cs = slice(half*512, half*512+512)
                    RR = psX.tile([64, 512], F32, tag="psx", name="RR")
                    nc.tensor.matmul(RR, ct['ONES1'], R1B[:, cs], start=True, stop=True)
                    nc.vector.tensor_mul(out=U2D[0:64, cs], in0=XCS[:, cs], in1=RR)
                nc.scalar.dma_start(out=U2D[64:128, 0:1023], in_=U2D[0:64, 1:1024])
                st['U2D'] = U2D
                P1S = spool.tile([64, 32], F32, tag="p1s", name="P1S")
                nc.vector.reduce_sum(out=P1S, in_=U2D[0:64, :].rearrange("p (a b) -> p a b", b=32),
                                     axis=mybir.AxisListType.X)
                USQ = spool.tile([64, 1024], F32, tag="usq", name="USQ")
                nc.vector.tensor_mul(out=USQ, in0=U2D[0:64, :], in1=U2D[0:64, :])
                P2S = spool.tile([64, 32], F32, tag="p2s", name="P2S")
                nc.vector.reduce_sum(out=P2S, in_=USQ.rearrange("p (a b) -> p a b", b=32),
                                     axis=mybir.AxisListType.X)
                P1B = spool.tile([64, 32], BF16, tag="p1b", name="P1B")
                P2B = spool.tile([64, 32], BF16, tag="p2b", name="P2B")
                nc.vector.tensor_copy(out=P1B, in_=P1S)
                nc.vector.tensor_copy(out=P2B, in_=P2S)
                STP = psX.tile([32, 2], F32, tag="psx", name="STP")
                nc.tensor.matmul(STP[:, 0:1], P1B, ct['M2048N'], start=True, stop=True)
                nc.tensor.matmul(STP[:, 1:2], P2B, ct['M2048P'], start=True, stop=True,
                                 skip_group_check=True)
                MUP = psX.tile([1, 32], F32, tag="psx", name="MUP")
                nc.tensor.matmul(MUP, ct['M2048N'], P1B, start=True, stop=True)
                MUNB = spool.tile([1, 32], BF16, tag="munb", bufs=3, name="MUNB")
                nc.vector.tensor_copy(out=MUNB, in_=MUP)
                STB = spool.tile([32, 2], F32, tag="stb", name="STB")
                nc.vector.tensor_copy(out=STB, in_=STP)
                MM2 = spool.tile([32, 1], F32, tag="mm2", name="MM2")
                nc.vector.tensor_mul(out=MM2, in0=STB[:, 0:1], in1=STB[:, 0:1])
                V32 = spool.tile([32, 1], F32, tag="v32", name="V32")
                nc.vector.tensor_sub(out=V32, in0=STB[:, 1:2], in1=MM2)
                SQ32 = spool.tile([32, 1], F32, tag="sq32", name="SQ32")
                nc.scalar.activation(out=SQ32, in_=V32, func=AF.Sqrt, bias=ct['XLB'])
                S4t = spool.tile([32, 1], F32, tag="s4", bufs=3, name="S4t")
                nc.vector.reciprocal_approx_fast(out=S4t, in_=SQ32)
                st['S4'] = S4t
                st['MUNB'] = MUNB

            def stageS5(m, st):
                U2D = st['U2D']; WT = st['WT']
                FO = psF.tile([32, 512], F32, tag="fo", name="FO")
                for k in range(16):
                    lhs = bass.AP(tensor=U2D.tensor, offset=U2D.offset + 2*k, ap=[U2D.ap[0], [32, 32]])
                    nc.tensor.matmul(FO, lhs, WT[:, k*512:(k+1)*512], start=(k == 0), stop=False)
                nc.tensor.matmul(FO, st['MUNB'], ct['W1SUM'][m], start=False, stop=True)
                nc.vector.scalar_tensor_tensor(out=FACC, in0=FO, scalar=st['S4'], in1=FACC,
                                               op0=ALU.mult, op1=ALU.add)

            states = {}
            for m in range(16):
                states[m] = stageS1(m)
                if m >= 1:
                    stageS2(m-1, states[m-1]); stageS3(m-1, states[m-1]); stageS4(m-1, states[m-1])
                if m >= 2:
                    stageS5(m-2, states[m-2]); del states[m-2]
            stageS2(15, states[15]); stageS3(15, states[15]); stageS4(15, states[15])
            stageS5(14, states[14]); stageS5(15, states[15])

            # ---------------- final MLP
            O1B = spool.tile([32, 512], BF16, tag="o1b")
            T5 = spool.tile([32, 512], F32, tag="t5")
            nc.vector.tensor_add(out=T5, in0=FACC, in1=ct['FB1R'])
            nc.vector.tensor_scalar_max(out=O1B, in0=T5, scalar1=0.0)
            F2 = psX.tile([32, 128], F32, tag="psx")
            for k in range(4):
                PL2 = psO.tile([128, 32], BF16, tag="pso")
                nc.tensor.transpose(PL2, O1B[:, k*128:(k+1)*128], IDT[0:32, 0:32])
                L2 = spool.tile([128, 32], BF16, tag="l2")
                nc.scalar.copy(out=L2, in_=PL2)
                nc.tensor.matmul(F2, L2, ct['FW2'][k], start=(k == 0), stop=(k == 3))
            O2B = spool.tile([32, 128], BF16, tag="o2b")
            T6 = spool.tile([32, 128], F32, tag="t6")
            nc.vector.tensor_add(out=T6, in0=F2, in1=ct['FB2R'])
            nc.vector.tensor_scalar_max(out=O2B, in0=T6, scalar1=0.0)
            PL3 = psO.tile([128, 32], BF16, tag="pso")
            nc.tensor.transpose(PL3, O2B, IDT[0:32, 0:32])
            L3 = spool.tile([128, 32], BF16, tag="l3")
            nc.scalar.copy(out=L3, in_=PL3)
            F3 = psX.tile([32, 25], F32, tag="psx")
            nc.tensor.matmul(F3, L3, ct['FW3'], start=True, stop=True)
            OUTS = spool.tile([32, 25], F32, tag="outs")
            nc.vector.tensor_add(out=OUTS, in0=F3, in1=ct['FB3R'])
            nc.gpsimd.dma_start(out=out_d[:, :], in_=OUTS)

    nc.compile()
    return nc


_NC_CACHE = None

def _get_nc():
    global _NC_CACHE
    if _NC_CACHE is None:
        _NC_CACHE = build_nc()
    return _NC_CACHE


def kernel(**inputs):
    nc = _get_nc()
    shared = host_prep_shared(inputs)
    in_maps = []
    for c in range(8):
        d = dict(shared)
        d.update(host_prep(inputs, c))
        in_maps.append(d)
    res = run_bass_kernel_spmd(nc, in_maps, list(range(8)))
    out = np.concatenate([res.results[c]["OUT"] for c in range(8)], axis=0)
    return out.astype(np.float32)


if __name__ == '__main__':
    import pickle
    with open('/tmp/refdata.pkl', 'rb') as f:
        inputs, expected = pickle.load(f)
    out = kernel(**inputs)
    rv = float(((out - expected)**2).mean() / (expected**2).mean())
    print("kernel resid_var:", rv)


# revision 15
# speedup vs baseline: 1.1755x; 1.0016x over previous
"""Trainium2 Bass kernel for nn_AttenModFullXL: 8-core data-parallel over batch.

v2: DMA batching + dual HWDGE queues, Pool-engine offload for pools/evictions,
PE transposes (no DMA transposes), fused expand bias via identity matmul,
batched softmax + fast reciprocal, collapsed XL-LN as FO post-scale with a
rank-1 mean-correction matmul, s-paired fl1 with one weight DMA per module.

Key structural facts exploited:
  - XL attention outputs are discarded by the reference (dead code) -> skipped.
  - Double XL LayerNorm collapses to one center + per-row scalar scale (exact):
    scale = 4*rsqrt((16+4e)*var + e^2), applied to fl1 partials per batch row.
  - conv1/conv2 run as block-diagonal matmuls over 4-image groups.
  - All matmul inputs bf16 (fp32 accumulate); elementwise/LN/softmax fp32.
"""
import sys, os
sys.path.insert(0, '/root/.axon_site/_ro/trn_rl_repo')
import numpy as np
import ml_dtypes

import concourse.bass as bass
import concourse.mybir as mybir
import concourse.tile as tile
from concourse import bacc
from concourse.bass_utils import run_bass_kernel_spmd
from concourse.masks import make_identity
from contextlib import ExitStack

F32 = mybir.dt.float32
BF16 = mybir.dt.bfloat16
AF = mybir.ActivationFunctionType
ALU = mybir.AluOpType
BF = ml_dtypes.bfloat16
EPS = 1e-5


# ---------------------------------------------------------------- host prep
def host_prep(inputs, c):
    t = np.asarray(inputs['t'], np.float32)
    bs = t.shape[0]
    patches = t.reshape(bs, 4, 32, 4, 32).transpose(0, 1, 3, 2, 4).reshape(bs, 16, 32, 32)
    bl = np.arange(32)
    m_i = np.arange(16)
    bidx = (16 * m_i[:, None] + 2 * c + (bl[None, :] >> 4)).reshape(-1)
    pidx = np.tile(bl & 15, 16)
    imgs = patches[bidx, pidx]                     # (512, 32, 32) f_local = m*32+bl
    sw = np.lib.stride_tricks.sliding_window_view(imgs, (3, 3), axis=(1, 2))
    i2c = sw.transpose(0, 3, 4, 1, 2).reshape(512, 9, 900)
    # (64 pairs, 36 = 4img x 9tap, 2 groups, 900): 2 groups side-by-side per row
    i2c = i2c.reshape(64, 2, 36, 900).transpose(0, 2, 1, 3)
    return {'IM2C': np.ascontiguousarray(i2c).astype(BF)}


def host_prep_shared(inputs):
    d = {}
    w1 = np.asarray(inputs['conv1_w'], np.float32) / 255.0
    # rows = 9i+tap (duplicated at partition 64), cols = 32i+ch
    W1BD = np.zeros((128, 128), np.float32)
    wt = w1[:, 0].reshape(32, 9).T                 # (9 taps, 32 ch)
    for i in range(4):
        W1BD[9*i:9*i+9, 32*i:32*i+32] = wt
        W1BD[64+9*i:64+9*i+9, 32*i:32*i+32] = wt
    d['W1BD'] = W1BD.astype(BF)

    w2 = np.asarray(inputs['conv2_w'], np.float32)
    W2BD = np.zeros((9, 128, 128), np.float32)
    for tap in range(9):
        blk = w2[:, :, tap // 3, tap % 3].T
        for i in range(4):
            W2BD[tap, 32*i:32*i+32, 32*i:32*i+32] = blk
    d['W2BD'] = W2BD.astype(BF)

    expw = np.asarray(inputs['exp_w'], np.float32)
    d['EXPW'] = expw.astype(BF)
    b2_eff = np.asarray(inputs['conv2_b'], np.float32) + \
        np.asarray(inputs['conv1_b'], np.float32) @ w2.sum(axis=(2, 3)).T
    expb_eff = np.asarray(inputs['exp_b'], np.float32)[None, :] + \
        b2_eff[:, None] * expw.sum(axis=0)[None, :]
    d['EXPB'] = expb_eff.astype(BF)                # (32 ch, 64) stationary for bias mm
    IDC = np.zeros((32, 128), np.float32)
    for i in range(4):
        IDC[:, 32*i:32*i+32] = np.eye(32)
    d['IDC'] = IDC.astype(BF)

    wq = np.asarray(inputs['mh_wq'], np.float32); bq = np.asarray(inputs['mh_bq'], np.float32)
    wk = np.asarray(inputs['mh_wk'], np.float32); bk = np.asarray(inputs['mh_bk'], np.float32)
    wv = np.asarray(inputs['mh_wv'], np.float32); bv = np.asarray(inputs['mh_bv'], np.float32)
    wo = np.asarray(inputs['mh_wo'], np.float32); bo = np.asarray(inputs['mh_bo'], np.float32)
    LQ = np.zeros((16, 65, 128), np.float32); LK = np.zeros((16, 65, 128), np.float32)
    LV = np.zeros((16, 65, 128), np.float32)
    WOP = np.zeros((16, 128, 64), np.float32)
    for m in range(16):
        for h in range(4):
            LQ[m, :64, 32*h:32*h+16] = wq[m, h] * 0.25
            LK[m, :64, 32*h:32*h+16] = wk[m, h]
            LV[m, :64, 32*h:32*h+16] = wv[m, h]
            LQ[m, 64, 32*h:32*h+16] = bq[m, h] * 0.25
            LK[m, 64, 32*h:32*h+16] = bk[m, h]
            LV[m, 64, 32*h:32*h+16] = bv[m, h]
            WOP[m, 32*h:32*h+16, :] = wo[m, 16*h:16*h+16, :]
    d['LQ'] = LQ.astype(BF); d['LK'] = LK.astype(BF); d['LV'] = LV.astype(BF)
    d['WOP'] = WOP.astype(BF)
    d['BO'] = bo.reshape(16, 64, 1).astype(np.float32)

    d['CEN'] = (np.eye(64, dtype=np.float32) - 1.0/64.0).astype(BF)
    d['MEAN64'] = np.full((64, 1), 1.0/64.0, np.float32).astype(BF)
    d['ONES1'] = np.ones((1, 64), np.float32).astype(BF)
    d['EPS1'] = np.full((1, 1), EPS, np.float32)
    d['M2048N'] = np.full((64, 1), -1.0/2048.0, np.float32).astype(BF)
    d['XLB'] = np.full((32, 1), EPS*EPS/16.0, np.float32)
    d['M2048P'] = np.full((64, 1), 1.0/2048.0, np.float32).astype(BF)

    fl1 = np.asarray(inputs['fl1_w'], np.float32)
    # [m, 64*(s%2)+d, (s//2)*512+f]
    arr = fl1.reshape(16, 16, 2, 64, 512)          # (m, s2, sp, d, f)
    d['FL1W'] = np.ascontiguousarray(arr.transpose(0, 2, 3, 1, 4).reshape(16, 128, 8192)).astype(BF)
    d['W1SUM'] = fl1.reshape(16, 2048, 512).sum(axis=1).reshape(16, 1, 512).astype(BF)
    d['FB1R'] = np.broadcast_to(np.asarray(inputs['fl1_b'], np.float32), (32, 512)).copy()
    d['FW2'] = np.asarray(inputs['fl2_w'], np.float32).reshape(4, 128, 128).astype(BF)
    d['FB2R'] = np.broadcast_to(np.asarray(inputs['fl2_b'], np.float32), (32, 128)).copy()
    d['FW3'] = np.asarray(inputs['fl3_w'], np.float32).astype(BF)
    d['FB3R'] = np.broadcast_to(np.asarray(inputs['fl3_b'], np.float32), (32, 25)).copy()
    return d


CONST_SPECS = [  # name, shape, dtype
    ('W1BD', (128, 128), BF16), ('W2BD', (9, 128, 128), BF16),
    ('EXPW', (36, 64), BF16), ('EXPB', (32, 64), BF16), ('IDC', (32, 128), BF16),
    ('LQ', (16, 65, 128), BF16), ('LK', (16, 65, 128), BF16), ('LV', (16, 65, 128), BF16),
    ('WOP', (16, 128, 64), BF16), ('BO', (16, 64, 1), F32),
    ('CEN', (64, 64), BF16), ('MEAN64', (64, 1), BF16), ('ONES1', (1, 64), BF16),
    ('EPS1', (1, 1), F32), ('M2048N', (64, 1), BF16), ('XLB', (32, 1), F32), ('M2048P', (64, 1), BF16),
    ('W1SUM', (16, 1, 512), BF16),
    ('FB1R', (32, 512), F32), ('FW2', (4, 128, 128), BF16),
    ('FB2R', (32, 128), F32), ('FW3', (128, 25), BF16), ('FB3R', (32, 25), F32),
]

C1P = 1.0 + EPS / 4.0        # scale for collapsed XL-LN rsqrt arg
C2P = EPS * EPS / 16.0


# ---------------------------------------------------------------- device build
def build_nc():
    nc = bacc.Bacc(None)
    im2c_d = nc.dram_tensor("IM2C", [64, 36, 2, 900], BF16, kind="ExternalInput")
    fl1w_d = nc.dram_tensor("FL1W", [16, 128, 8192], BF16, kind="ExternalInput")
    cd = {}
    for name, shape, dt in CONST_SPECS:
        cd[name] = nc.dram_tensor(name, list(shape), dt, kind="ExternalInput")
    out_d = nc.dram_tensor("OUT", [32, 25], F32, kind="ExternalOutput")

    with tile.TileContext(nc) as tc, ExitStack() as top:
        cpool = top.enter_context(tc.tile_pool(name="consts", bufs=1))
        CONV_CONSTS = {'W1BD', 'W2BD', 'EXPW', 'EXPB', 'IDC'}
        ct = {}
        deferred = []
        qtoggle = [0]
        def load_const(name, shape, dt):
            if len(shape) == 3:
                tiles = []
                for k in range(shape[0]):
                    til = cpool.tile(list(shape[1:]), dt, tag=f"c_{name}_{k}", name=f"c_{name}_{k}")
                    eng = nc.sync if qtoggle[0] % 2 == 0 else nc.scalar
                    qtoggle[0] += 1
                    eng.dma_start(out=til, in_=cd[name][k])
                    tiles.append(til)
                ct[name] = tiles
            else:
                til = cpool.tile(list(shape), dt, tag=f"c_{name}", name=f"c_{name}")
                eng = nc.sync if qtoggle[0] % 2 == 0 else nc.scalar
                qtoggle[0] += 1
                eng.dma_start(out=til, in_=cd[name][:, :])
                ct[name] = til
        for name, shape, dt in CONST_SPECS:
            if name not in CONV_CONSTS:
                deferred.append((name, shape, dt))
        IDT = cpool.tile([128, 128], BF16, tag="idt")
        make_identity(nc, IDT)
        U_ALL = cpool.tile([65, 16384], BF16, tag="uall")
        nc.vector.memset(U_ALL[64:65, :], 1.0)
        FACC = cpool.tile([32, 512], F32, tag="facc")
        nc.vector.memset(FACC, 0.0)

        # ---------------- stage A: conv stack -> U_ALL
        with ExitStack() as sa:
            rpool = sa.enter_context(tc.tile_pool(name="sa_rhs", bufs=4))
            wpool = sa.enter_context(tc.tile_pool(name="sa_w", bufs=3))
            dpool = sa.enter_context(tc.tile_pool(name="sa_d", bufs=2))
            psA = sa.enter_context(tc.tile_pool(name="psA", bufs=2, space="PSUM"))
            psB = sa.enter_context(tc.tile_pool(name="psB", bufs=2, space="PSUM"))
            psT = sa.enter_context(tc.tile_pool(name="psT", bufs=1, space="PSUM"))
            psE = sa.enter_context(tc.tile_pool(name="psE", bufs=1, space="PSUM"))
            def stage_conv1(rt, base, gi):
                PA = psA.tile([128, 1024], F32, tag="pa")
                rg = rt[base:base+36, gi*900:gi*900+900]
                nc.tensor.matmul(PA[:, 0:480], ct['W1BD'][base:base+36, :], rg[:, 0:480], start=True, stop=True)
                nc.tensor.matmul(PA[:, 512:932], ct['W1BD'][base:base+36, :], rg[:, 480:900], start=True, stop=True)
                return PA

            def stage_conv2(PA):
                # pool1: x-max (copies on scalar, maxes on DVE), then y-max
                M1 = wpool.tile([128, 450], F32, tag="m1")
                r1a = bass.AP(tensor=PA.tensor, offset=PA.offset, ap=[PA.ap[0], [30, 16], [2, 15]])
                r1b = bass.AP(tensor=PA.tensor, offset=PA.offset + 1, ap=[PA.ap[0], [30, 16], [2, 15]])
                m1a = M1[:, 0:240].rearrange("p (a b) -> p a b", b=15)
                nc.scalar.copy(out=m1a, in_=r1a)
                nc.vector.tensor_max(out=m1a, in0=m1a, in1=r1b)
                r2a = bass.AP(tensor=PA.tensor, offset=PA.offset + 512, ap=[PA.ap[0], [30, 14], [2, 15]])
                r2b = bass.AP(tensor=PA.tensor, offset=PA.offset + 513, ap=[PA.ap[0], [30, 14], [2, 15]])
                m1b = M1[:, 240:450].rearrange("p (a b) -> p a b", b=15)
                nc.scalar.copy(out=m1b, in_=r2a)
                nc.vector.tensor_max(out=m1b, in0=m1b, in1=r2b)
                U1 = wpool.tile([128, 225], BF16, tag="u1")
                ya = bass.AP(tensor=M1.tensor, offset=M1.offset, ap=[M1.ap[0], [30, 15], [1, 15]])
                yb = bass.AP(tensor=M1.tensor, offset=M1.offset + 15, ap=[M1.ap[0], [30, 15], [1, 15]])
                nc.vector.tensor_max(out=U1[:, :].rearrange("p (a b) -> p a b", b=15), in0=ya, in1=yb)
                PB = psB.tile([128, 156], F32, tag="pb")
                for tap in range(9):
                    off = (tap // 3) * 15 + (tap % 3)
                    v = bass.AP(tensor=U1.tensor, offset=U1.offset + off,
                                ap=[U1.ap[0], [15, 12], [1, 13]])
                    nc.tensor.matmul(PB, ct['W2BD'][tap], v,
                                     start=(tap == 0), stop=(tap == 8))
                return PB

            def stage_tail(PB, g):
                M2 = wpool.tile([128, 72], F32, tag="m2")
                xa = bass.AP(tensor=PB.tensor, offset=PB.offset, ap=[PB.ap[0], [13, 12], [2, 6]])
                xb = bass.AP(tensor=PB.tensor, offset=PB.offset + 1, ap=[PB.ap[0], [13, 12], [2, 6]])
                m2v = M2[:, :].rearrange("p (a b) -> p a b", b=6)
                nc.scalar.copy(out=m2v, in_=xa)
                nc.vector.tensor_max(out=m2v, in0=m2v, in1=xb)
                U2T = wpool.tile([128, 36], BF16, tag="u2t")
                ka = bass.AP(tensor=M2.tensor, offset=M2.offset, ap=[M2.ap[0], [12, 6], [1, 6]])
                kb = bass.AP(tensor=M2.tensor, offset=M2.offset + 6, ap=[M2.ap[0], [12, 6], [1, 6]])
                nc.vector.tensor_max(out=U2T[:, :].rearrange("p (a b) -> p a b", b=6), in0=ka, in1=kb)
                PT = psT.tile([36, 128], BF16, tag="pt")
                nc.tensor.transpose(PT, U2T, IDT)
                TPS = dpool.tile([36, 128], BF16, tag="tps")
                nc.scalar.copy(out=TPS, in_=PT)
                PE1 = psE.tile([64, 128], F32, tag="pe1")
                nc.tensor.matmul(PE1, ct['EXPW'], TPS, start=True, stop=False)
                nc.tensor.matmul(PE1, ct['EXPB'], ct['IDC'], start=False, stop=True,
                                 skip_group_check=True)
                nc.vector.tensor_scalar_max(out=U_ALL[0:64, g*128:(g+1)*128], in0=PE1, scalar1=0.0)

            # 3-stage software pipeline over the 128 groups
            def load_quad(quad):
                rt = rpool.tile([128, 1800], BF16, tag="im2c", name="rt")
                nc.sync.dma_start(out=rt[0:36, :], in_=im2c_d[2*quad])
                nc.scalar.dma_start(out=rt[64:100, :], in_=im2c_d[2*quad+1])
                return rt
            rts = {0: load_quad(0), 1: load_quad(1)}
            for name, shape, dt in CONST_SPECS:
                if name in CONV_CONSTS:
                    load_const(name, shape, dt)
            q1 = []   # (PA, g) awaiting stage_conv2
            q2 = []   # (PB, g) awaiting stage_tail
            for quad in range(32):
                rt = rts.pop(quad)
                if quad + 2 < 32 and quad + 2 not in rts:
                    rts[quad + 2] = load_quad(quad + 2)
                for sub in range(4):
                    g = 4*quad + sub
                    PA = stage_conv1(rt, 64 * (sub // 2), sub % 2)
                    q1.append((PA, g))
                    if len(q1) > 1:
                        PAp, gp = q1.pop(0)
                        q2.append((stage_conv2(PAp), gp))
                    if len(q2) > 1:
                        PBp, gp = q2.pop(0)
                        stage_tail(PBp, gp)
            while q1:
                PAp, gp = q1.pop(0)
                q2.append((stage_conv2(PAp), gp))
            while q2:
                PBp, gp = q2.pop(0)
                stage_tail(PBp, gp)
            for name, shape, dt in deferred:
                load_const(name, shape, dt)

        # ---------------- stage B: per-module attention + LN + fl1 accumulation
        with ExitStack() as sbk:
            apool = sbk.enter_context(tc.tile_pool(name="sb_sb", bufs=2))
            spool = sbk.enter_context(tc.tile_pool(name="sb_small", bufs=2))
            atpool = sbk.enter_context(tc.tile_pool(name="sb_at", bufs=4))
            w1pool = sbk.enter_context(tc.tile_pool(name="w1t", bufs=2))
            psP = sbk.enter_context(tc.tile_pool(name="psP", bufs=2, space="PSUM"))
            psO = sbk.enter_context(tc.tile_pool(name="psO", bufs=3, space="PSUM"))
            psX = sbk.enter_context(tc.tile_pool(name="psX", bufs=2, space="PSUM"))
            psF = sbk.enter_context(tc.tile_pool(name="psF", bufs=1, space="PSUM"))
            def stageS1(m):
                st = {}
                WT = w1pool.tile([128, 8192], BF16, tag="w1t", bufs=3, name="WT")
                weng = nc.sync if (m % 2 == 0) else nc.scalar
                weng.dma_start(out=WT, in_=fl1w_d[m])
                st['WT'] = WT
                QKT = apool.tile([128, 2048], BF16, tag="qkt", name="QKT")
                for half in range(2):
                    cs = slice(half*512, half*512+512)
                    PQ = psP.tile([128, 512], F32, tag="psp", name="PQ")
                    nc.tensor.matmul(PQ, ct['LQ'][m], U_ALL[:, m*1024+half*512:m*1024+half*512+512],
                                     start=True, stop=True)
                    nc.scalar.copy(out=QKT[:, cs], in_=PQ)
                    PK = psP.tile([128, 512], F32, tag="psp", name="PK")
                    nc.tensor.matmul(PK, ct['LK'][m], U_ALL[:, m*1024+half*512:m*1024+half*512+512],
                                     start=True, stop=True)
                    nc.vector.tensor_copy(out=QKT[:, 1024+half*512:1024+half*512+512], in_=PK)
                eas = []
                for half in range(2):
                    SC = psP.tile([128, 512], F32, tag="psp", name="SC")
                    for bj in range(16):
                        b = half * 16 + bj
                        for h in range(4):
                            nc.tensor.matmul(
                                SC[32*h:32*h+32, bj*32:bj*32+32],
                                QKT[32*h:32*h+16, b*32:b*32+32],
                                QKT[32*h:32*h+16, 1024+b*32:1024+b*32+32],
                                start=True, stop=True, tile_position=(32*h, 32*h))
                    EA = spool.tile([128, 512], F32, tag="ea", bufs=4, name="EA")
                    nc.scalar.activation(out=EA, in_=SC, func=AF.Exp)
                    eas.append(EA)
                st['eas'] = eas
                return st

            def stageS2(m, st):
                A_ALL = apool.tile([128, 1024], BF16, tag="aall", name="A_ALL")
                for half in range(2):
                    EA = st['eas'][half]
                    SM = spool.tile([128, 16], F32, tag="sm", name="SM")
                    nc.vector.reduce_sum(out=SM, in_=EA.rearrange("p (a b) -> p a b", b=32),
                                         axis=mybir.AxisListType.X)
                    RS = spool.tile([128, 16], F32, tag="rs", name="RS")
                    nc.vector.reciprocal_approx_fast(out=RS, in_=SM)
                    rsb = bass.AP(tensor=RS.tensor, offset=RS.offset, ap=[RS.ap[0], [1, 16], [0, 32]])
                    nc.vector.tensor_tensor(
                        out=A_ALL[:, half*512:(half+1)*512].rearrange("p (a b) -> p a b", b=32),
                        in0=EA.rearrange("p (a b) -> p a b", b=32), in1=rsb, op=ALU.mult)
                st['A_ALL'] = A_ALL

            def stageS3(m, st):
                A_ALL = st['A_ALL']
                OT = apool.tile([128, 1024], BF16, tag="ot", name="OT")
                for B in range(8):
                    PV = psO.tile([128, 128], F32, tag="pso", name="PV")
                    for j in range(4):
                        bp = 4*B + j
                        nc.tensor.matmul(PV[32*j:32*j+32, :],
                                         U_ALL[:, m*1024+bp*32:m*1024+bp*32+32], ct['LV'][m],
                                         start=True, stop=True, tile_position=(0, 32*j))
                    VN = atpool.tile([128, 128], BF16, tag="vn", name="VN")
                    nc.scalar.copy(out=VN, in_=PV)
                    PTA = psO.tile([128, 128], BF16, tag="pso", name="PTA")
                    nc.tensor.transpose(PTA, A_ALL[:, B*128:(B+1)*128], IDT)
                    ATS = atpool.tile([128, 128], BF16, tag="ats", name="ATS")
                    nc.scalar.copy(out=ATS, in_=PTA)
                    for j in range(4):
                        POj = psO.tile([128, 32], F32, tag="pso", name="POj")
                        for h in range(4):
                            nc.tensor.matmul(
                                POj[32*h:32*h+32, 0:32],
                                VN[32*j:32*j+32, 32*h:32*h+32],
                                ATS[32*j:32*j+32, 32*h:32*h+32],
                                start=True, stop=True, tile_position=(32*j, 32*h),
                                skip_group_check=True)
                        if j % 2 == 0:
                            nc.scalar.copy(out=OT[:, B*128+j*32:B*128+j*32+32], in_=POj)
                        else:
                            nc.vector.tensor_copy(out=OT[:, B*128+j*32:B*128+j*32+32], in_=POj)
                st['OT'] = OT

            def stageS4a(m, st):
                useg = U_ALL[0:64, m*1024:(m+1)*1024]
                OT = st['OT']
                XB = apool.tile([64, 1024], BF16, tag="xb", name="XB")
                for half in range(2):
                    cs = slice(half*512, half*512+512)
                    XO = psX.tile([64, 512], F32, tag="psx", name="XO")
                    nc.tensor.matmul(XO, ct['WOP'][m], OT[:, cs], start=True, stop=True)
                    nc.vector.scalar_tensor_tensor(out=XB[:, cs], in0=XO, scalar=ct['BO'][m],
                                                   in1=useg[:, cs], op0=ALU.add, op1=ALU.add)
                st['XB'] = XB

            def stageS4b(m, st):
                XB = st['XB']
                XCS = apool.tile([64, 1024], BF16, tag="xcs", name="XCS")
                XSQ = apool.tile([64, 1024], BF16, tag="xsq", name="XSQ")
                SQV = spool.tile([1, 1024], F32, tag="sqv", name="SQV")
                for half in range(2):
                    cs = slice(half*512, half*512+512)
                    XC = psX.tile([64, 512], F32, tag="psx", name="XC")
                    nc.tensor.matmul(XC, ct['CEN'], XB[:, cs], start=True, stop=True)
                    nc.vector.tensor_copy(out=XCS[:, cs], in_=XC)
                    nc.vector.tensor_mul(out=XSQ[:, cs], in0=XCS[:, cs], in1=XCS[:, cs])
                    V1 = psX.tile([1, 512], F32, tag="psx", name="V1")
                    nc.tensor.matmul(V1, ct['MEAN64'], XSQ[:, cs], start=True, stop=True)
                    nc.scalar.activation(out=SQV[:, cs], in_=V1, func=AF.Sqrt, bias=ct['EPS1'])
                R1F = spool.tile([1, 1024], F32, tag="r1f", name="R1F")
                nc.vector.reciprocal_approx_fast(out=R1F, in_=SQV)
                R1B = spool.tile([1, 1024], BF16, tag="r1b", name="R1B")
                nc.vector.tensor_copy(out=R1B, in_=R1F)
                U2D = apool.tile([128, 1024], BF16, tag="u2d", bufs=3, name="U2D")
                for half in range(2):
                    cs = slice(half*512, half*512+512)
                    RR = psX.tile([64, 512], F32, tag="psx", name="RR")
                    nc.tensor.matmul(RR, ct['ONES1'], R1B[:, cs], start=True, stop=True)
                    nc.vector.tensor_mul(out=U2D[0:64, cs], in0=XCS[:, cs], in1=RR)
                nc.scalar.dma_start(out=U2D[64:128, 0:1023], in_=U2D[0:64, 1:1024])
                st['U2D'] = U2D

            def stageS4c(m, st):
                U2D = st['U2D']
                P1S = spool.tile([64, 32], F32, tag="p1s", name="P1S")
                nc.vector.reduce_sum(out=P1S, in_=U2D[0:64, :].rearrange("p (a b) -> p a b", b=32),
                                     axis=mybir.AxisListType.X)
                USQ = spool.tile([64, 1024], F32, tag="usq", name="USQ")
                nc.vector.tensor_mul(out=USQ, in0=U2D[0:64, :], in1=U2D[0:64, :])
                P2S = spool.tile([64, 32], F32, tag="p2s", name="P2S")
                nc.vector.reduce_sum(out=P2S, in_=USQ.rearrange("p (a b) -> p a b", b=32),
                                     axis=mybir.AxisListType.X)
                P1B = spool.tile([64, 32], BF16, tag="p1b", name="P1B")
                P2B = spool.tile([64, 32], BF16, tag="p2b", name="P2B")
                nc.vector.tensor_copy(out=P1B, in_=P1S)
                nc.vector.tensor_copy(out=P2B, in_=P2S)
                STP = psX.tile([32, 2], F32, tag="psx", name="STP")
                nc.tensor.matmul(STP[:, 0:1], P1B, ct['M2048N'], start=True, stop=True)
                nc.tensor.matmul(STP[:, 1:2], P2B, ct['M2048P'], start=True, stop=True,
                                 skip_group_check=True)
                MUP = psX.tile([1, 32], F32, tag="psx", name="MUP")
                nc.tensor.matmul(MUP, ct['M2048N'], P1B, start=True, stop=True)
                MUNB = spool.tile([1, 32], BF16, tag="munb", bufs=3, name="MUNB")
                nc.vector.tensor_copy(out=MUNB, in_=MUP)
                STB = spool.tile([32, 2], F32, tag="stb", name="STB")
                nc.vector.tensor_copy(out=STB, in_=STP)
                MM2 = spool.tile([32, 1], F32, tag="mm2", name="MM2")
                nc.vector.tensor_mul(out=MM2, in0=STB[:, 0:1], in1=STB[:, 0:1])
                V32 = spool.tile([32, 1], F32, tag="v32", name="V32")
                nc.vector.tensor_sub(out=V32, in0=STB[:, 1:2], in1=MM2)
                SQ32 = spool.tile([32, 1], F32, tag="sq32", name="SQ32")
                nc.scalar.activation(out=SQ32, in_=V32, func=AF.Sqrt, bias=ct['XLB'])
                S4t = spool.tile([32, 1], F32, tag="s4", bufs=3, name="S4t")
                nc.vector.reciprocal_approx_fast(out=S4t, in_=SQ32)
                st['S4'] = S4t
                st['MUNB'] = MUNB

            def stageS5a(m, st):
                U2D = st['U2D']; WT = st['WT']
                FO = psF.tile([32, 512], F32, tag="fo", name="FO")
                for k in range(8):
                    lhs = bass.AP(tensor=U2D.tensor, offset=U2D.offset + 2*k, ap=[U2D.ap[0], [32, 32]])
                    nc.tensor.matmul(FO, lhs, WT[:, k*512:(k+1)*512], start=(k == 0), stop=False)
                st['FO'] = FO

            def stageS5b(m, st):
                U2D = st['U2D']; WT = st['WT']; FO = st['FO']
                for k in range(8, 16):
                    lhs = bass.AP(tensor=U2D.tensor, offset=U2D.offset + 2*k, ap=[U2D.ap[0], [32, 32]])
                    nc.tensor.matmul(FO, lhs, WT[:, k*512:(k+1)*512], start=False, stop=False)
                nc.tensor.matmul(FO, st['MUNB'], ct['W1SUM'][m], start=False, stop=True)
                nc.vector.scalar_tensor_tensor(out=FACC, in0=FO, scalar=st['S4'], in1=FACC,
                                               op0=ALU.mult, op1=ALU.add)

            def tail_stages(m, states):
                stageS2(m, states[m]); stageS3(m, states[m])
                stageS4a(m, states[m])
                if m >= 1: stageS5a(m-1, states[m-1])
                stageS4b(m, states[m])
                if m >= 1: stageS5b(m-1, states[m-1]); del states[m-1]
                stageS4c(m, states[m])

            states = {}
            for m in range(16):
                states[m] = stageS1(m)
                if m >= 1:
                    tail_stages(m-1, states)
            tail_stages(15, states)
            stageS5a(15, states[15]); stageS5b(15, states[15])

            # ---------------- final MLP
            O1B = spool.tile([32, 512], BF16, tag="o1b")
            T5 = spool.tile([32, 512], F32, tag="t5")
            nc.vector.tensor_add(out=T5, in0=FACC, in1=ct['FB1R'])
            nc.vector.tensor_scalar_max(out=O1B, in0=T5, scalar1=0.0)
            F2 = psX.tile([32, 128], F32, tag="psx")
            for k in range(4):
                PL2 = psO.tile([128, 32], BF16, tag="pso")
                nc.tensor.transpose(PL2, O1B[:, k*128:(k+1)*128], IDT[0:32, 0:32])
                L2 = spool.tile([128, 32], BF16, tag="l2")
                nc.scalar.copy(out=L2, in_=PL2)
                nc.tensor.matmul(F2, L2, ct['FW2'][k], start=(k == 0), stop=(k == 3))
            O2B = spool.tile([32, 128], BF16, tag="o2b")
            T6 = spool.tile([32, 128], F32, tag="t6")
            nc.vector.tensor_add(out=T6, in0=F2, in1=ct['FB2R'])
            nc.vector.tensor_scalar_max(out=O2B, in0=T6, scalar1=0.0)
            PL3 = psO.tile([128, 32], BF16, tag="pso")
            nc.tensor.transpose(PL3, O2B, IDT[0:32, 0:32])
            L3 = spool.tile([128, 32], BF16, tag="l3")
            nc.scalar.copy(out=L3, in_=PL3)
            F3 = psX.tile([32, 25], F32, tag="psx")
            nc.tensor.matmul(F3, L3, ct['FW3'], start=True, stop=True)
            OUTS = spool.tile([32, 25], F32, tag="outs")
            nc.vector.tensor_add(out=OUTS, in0=F3, in1=ct['FB3R'])
            nc.gpsimd.dma_start(out=out_d[:, :], in_=OUTS)

    nc.compile()
    return nc


_NC_CACHE = None

def _get_nc():
    global _NC_CACHE
    if _NC_CACHE is None:
        _NC_CACHE = build_nc()
    return _NC_CACHE


def kernel(**inputs):
    nc = _get_nc()
    shared = host_prep_shared(inputs)
    in_maps = []
    for c in range(8):
        d = dict(shared)
        d.update(host_prep(inputs, c))
        in_maps.append(d)
    res = run_bass_kernel_spmd(nc, in_maps, list(range(8)))
    out = np.concatenate([res.results[c]["OUT"] for c in range(8)], axis=0)
    return out.astype(np.float32)


if __name__ == '__main__':
    import pickle
    with open('/tmp/refdata.pkl', 'rb') as f:
        inputs, expected = pickle.load(f)
    out = kernel(**inputs)
    rv = float(((out - expected)**2).mean() / (expected**2).mean())
    print("kernel resid_var:", rv)
